# revision 73
# baseline (speedup 1.0000x reference)
"""Causal self-attention (B=4, T=2048, C=1024, H=16) on 8 trn2 NeuronCores.

Sharding: head-pair parallel. Core c owns heads {2c, 2c+1} for all 4 batches.
 - host: x is pre-transposed to xT [C, B*T] (bf16); W_qkv is packed per core
   into wqkv [C, 3*128] (2 heads x 64 each for q/k/v, softmax scale folded
   into the q columns), W_proj and biases broadcast. All matmul operands are
   bf16 (fp32 accumulate in PSUM); final output f32.
 - device per core: qkv projections as bf16 matmuls producing qT/kT/vT
   [d2, T] (bias added on DVE); vT is PE-transposed into per-head v [T, 65]
   tiles (64 dims + a ones column).
 - attention in S^T orientation: S^T[tk, tq] = kT.T@qT tiles [128, 512].
   Only the triangular 128-col block of each diagonal tile needs a causal
   mask: it is preloaded into PSUM via an identity matmul and the S matmul
   is split around it. Softmax without max-subtraction (|S| <= ~20 is safe
   in fp32): P^T = exp(S^T) on ScalarE (PSUM->SBUF bf16). The O-matmul
   lhsT = [v_h | ones] (M=65) yields O^T[d, tq] and the denominator row l
   in one pass.
 - softmax normalization per chunk: [65,512] PSUM->SBUF copy, [1,N]
   reciprocal of the denominator row, 1/l broadcast across 64 partitions
   via a K=1 ones-matmul on the PE, gpsimd multiply; normalized O^T ships
   through the per-batch AllToAll ([8, 128, 256] bf16) from head-shards to
   token-shards.
 - column-parallel out-projection with DVE-fused bias produces out^T
   [C, 1024 tokens] per core; host reassembles.
 - emission is software-pipelined: within each batch's attention loop,
   S(t+1) is emitted before O(t) (so the PE never queues behind the exp),
   and the next batch's qkv/v-transpose matmuls plus the previous batch's
   projection are drained as filler thunks between attention tiles. x
   chunks are prefetched a full batch ahead with single merged DMAs, and
   the AllToAll stand-in/collective is issued from the gpsimd queue so it
   never head-of-line-blocks the SP DMA queue.
"""
import numpy as np
import concourse.bacc as bacc
import concourse.mybir as mybir
import concourse.tile as tile
from concourse.bass_utils import run_bass_kernel_spmd
from concourse.masks import make_identity

F32 = mybir.dt.float32
BF16 = mybir.dt.bfloat16
Exp = mybir.ActivationFunctionType.Exp
Ident = mybir.ActivationFunctionType.Identity

NCORES = 8
B, T, C, H = 4, 2048, 1024, 16
HD = C // H          # 64
HL = H // NCORES     # 2 heads per core
D2 = HL * HD         # 128 rows of local head-pair dims
TB = T               # tokens per batch
NKC = C // 128       # 8 contraction chunks
NCH = TB // 512      # 4 tq chunks per batch
NTK = TB // 128      # 16 tk tiles per batch
PIECE = TB // NCORES  # 256 tokens per (batch, core) piece after AllToAll

_CACHE = {}


def _build(sim=False):
    nc = bacc.Bacc("TRN2", target_bir_lowering=False, debug=False,
                   num_devices=1 if sim else NCORES)
    xt = nc.dram_tensor("xt", [C, B * T], BF16, kind="ExternalInput").ap()
    wqkv = nc.dram_tensor("wqkv", [C, 3 * D2], BF16, kind="ExternalInput").ap()
    wp = nc.dram_tensor("wp", [C, C], BF16, kind="ExternalInput").ap()
    bqkv = nc.dram_tensor("bqkv", [D2, 3], F32, kind="ExternalInput").ap()
    bp = nc.dram_tensor("bp", [128, NKC], F32, kind="ExternalInput").ap()
    outp = nc.dram_tensor("outp", [C, B * PIECE], F32, kind="ExternalOutput").ap()

    inb = [nc.dram_tensor(f"inb{b}", [NCORES, D2, PIECE], BF16) for b in range(B)]
    outb = [nc.dram_tensor(f"outb{b}", [NCORES, D2, PIECE], BF16) for b in range(B)]

    with tile.TileContext(nc, pool_alloc_mode="queue") as tc:
        with (
            tc.tile_pool(name="const", bufs=1) as cpool,
            tc.tile_pool(name="w", bufs=1) as wpool,
            tc.tile_pool(name="xt", bufs=6) as xpool,
            tc.tile_pool(name="qk", bufs=2) as qkpool,
            tc.tile_pool(name="vstg", bufs=1) as vstgpool,
            tc.tile_pool(name="vh", bufs=2) as vhpool,
            tc.tile_pool(name="pt", bufs=5) as ptpool,
            tc.tile_pool(name="ofin", bufs=6) as ofinpool,
            tc.tile_pool(name="proj", bufs=2) as projpool,
            tc.tile_pool(name="otp", bufs=9) as otpool,
            tc.tile_pool(name="mm", bufs=2, space="PSUM") as mmps,
            tc.tile_pool(name="s", bufs=2, space="PSUM") as sps,
            tc.tile_pool(name="o", bufs=1, space="PSUM") as ops,
        ):
            # ---- constants (before weights so the weight/x DMAs queue
            # behind nothing heavy) ----
            ident32 = cpool.tile([128, 128], F32)
            make_identity(nc, ident32[:])
            idb = cpool.tile([128, 128], BF16)
            tri32 = cpool.tile([128, 128], F32)
            tri = cpool.tile([128, 128], BF16)
            ones32 = cpool.tile([128, 16], BF16)
            onesr = cpool.tile([1, 64], BF16)
            nc.gpsimd.memset(ones32[:], 1.0)
            nc.gpsimd.memset(onesr[:], 1.0)
            nc.gpsimd.memset(tri32[:], 0.0)
            # triangular block: keep where col >= row
            nc.gpsimd.affine_select(
                out=tri32[:], in_=tri32[:],
                compare_op=mybir.AluOpType.is_ge, fill=-1e30,
                base=0, channel_multiplier=-1,
                pattern=[[1, 128]],
            )
            with nc.allow_low_precision(reason="bf16 operand staging"):
                nc.vector.tensor_copy(idb[:], ident32[:])
                nc.vector.tensor_copy(tri[:], tri32[:])

            # ---- weights & first x chunks (order matters: everything the
            # first qkv matmuls need comes first) ----
            bqkv_sb = cpool.tile([D2, 3], F32)
            nc.sync.dma_start(bqkv_sb[:], bqkv)
            wqkv_sb = wpool.tile([128, NKC, 3, D2], BF16)
            wqr = wqkv.rearrange("(kc p) (g m) -> p kc g m", p=128, g=3)
            # split so the first qkv matmuls (kc=0,1) start sooner
            nc.sync.dma_start(wqkv_sb[:, 0:2], wqr[:, 0:2])
            nc.sync.dma_start(wqkv_sb[:, 2:NKC], wqr[:, 2:NKC])

            # x chunk prefetch: one merged DMA per 512-token chunk, issued a
            # full batch ahead so qkv never waits on HBM.
            xtr = xt.rearrange("(kc p) t -> p kc t", p=128)
            xq = {}

            def load_chunk(g):
                if g >= B * NCH or g in xq:
                    return
                t = xpool.tile([128, NKC, 512], BF16, tag="x")
                if g == 0:
                    # split the very first chunk so the first qkv matmuls
                    # start after a quarter of the transfer
                    nc.sync.dma_start(t[:, 0:2, :], xtr[:, 0:2, 0:512])
                    nc.sync.dma_start(t[:, 2:NKC, :], xtr[:, 2:NKC, 0:512])
                else:
                    nc.sync.dma_start(t[:], xtr[:, :, 512 * g:512 * (g + 1)])
                xq[g] = t

            for g in range(NCH):
                load_chunk(g)

            # needed only from the first projection (~1 batch in)
            bp_sb = cpool.tile([128, NKC], F32)
            nc.sync.dma_start(bp_sb[:], bp)
            wp_sb = wpool.tile([128, NKC, C], BF16)
            nc.sync.dma_start(
                wp_sb[:], wp.rearrange("(kc p) m -> p kc m", p=128))

            # per-batch qkv output tiles, kept across the interleaved emission
            qkv_tiles = {}

            def get_qkv(b):
                if b not in qkv_tiles:
                    qkv_tiles[b] = (
                        qkpool.tile([D2, TB], BF16, tag="qT", name=f"qT{b}"),
                        qkpool.tile([D2, TB], BF16, tag="kT", name=f"kT{b}"),
                        vstgpool.tile([D2, TB], BF16, tag="vT", name=f"vT{b}"),
                        [vhpool.tile([128, NTK * 65], BF16, tag=f"vh{h}",
                                     name=f"vh{h}_{b}") for h in range(HL)],
                    )
                return qkv_tiles[b]

            def qkv_thunks(g):
                """Emission closures for qkv chunk g=(b,n) + its v
                transposes, sized ~0.5-1.7us of PE work each so they can
                fill exp-latency bubbles inside the attention loop."""
                b, n = divmod(g, NCH)
                qT, kT, vT, vh = get_qkv(b)
                out = []

                # each col is split into two 4-matmul halves so filler
                # insertions between attention tiles stay ~0.9us
                ps_box = {}

                def mk_col(col, lo, hi):
                    def f():
                        xtile = xq[g]
                        if col == 0 and lo == 0:
                            load_chunk(g + NCH)
                        if lo == 0:
                            ps_box[col] = mmps.tile([128, 512], F32,
                                                    tag="ps", name="qkv_ps")
                        ps = ps_box[col]
                        for kc in range(lo, hi):
                            nc.tensor.matmul(
                                ps[:], wqkv_sb[:, kc, col, :],
                                xtile[:, kc, :], start=(kc == 0),
                                stop=(kc == NKC - 1))
                        if hi == NKC:
                            dst = (qT, kT, vT)[col]
                            with nc.allow_low_precision(reason="bf16 qkv"):
                                nc.vector.tensor_scalar_add(
                                    dst[:, 512 * n:512 * (n + 1)], ps[:],
                                    bqkv_sb[:, col:col + 1])
                            del ps_box[col]
                            if col == 2:
                                del xq[g]
                    return f

                for col in range(3):
                    for kc in range(NKC):
                        out.append(mk_col(col, kc, kc + 1))

                def mk_vt(h):
                    def f():
                        if n == 0:
                            nc.vector.tensor_copy(vh[h][:, 64::65], ones32[:])
                        tg = n
                        vt_ps = mmps.tile([128, 256], BF16, tag="ps",
                                          name="vt_ps")
                        for i in range(4):
                            tk = 4 * tg + i
                            nc.tensor.transpose(
                                vt_ps[:, 64 * i:64 * (i + 1)],
                                vT[64 * h:64 * (h + 1),
                                   128 * tk:128 * (tk + 1)],
                                idb[64 * h:64 * (h + 1), 64 * h:64 * (h + 1)])
                        dst = vh[h][:, 65 * 4 * tg:65 * 4 * (tg + 1)]
                        nc.vector.tensor_copy(
                            dst.rearrange("p (t c) -> p t c", t=4)[:, :, 0:64],
                            vt_ps[:].rearrange("p (t c) -> p t c", t=4))
                    return f

                for h in range(HL):
                    out.append(mk_vt(h))
                return out

            def proj_thunks(b):
                """Out-projection of batch b (column-parallel, out^T),
                consuming the AllToAll result one batch late. The softmax
                scale r arrives with the payload; Pool bcast + DVE multiply.
                The last batch ships pre-normalized O (sender-side scale)
                and stores per-mcol so the tail is as short as possible."""
                last = b == B - 1
                ots = []
                stg = projpool.tile([128, NKC, PIECE], F32, tag="stg")
                out = []

                def head():
                    for s8 in range(NCORES):
                        ot = otpool.tile([128, PIECE], BF16, tag="ot",
                                         name="ot")
                        nc.sync.dma_start(ot[:], outb[b].ap()[s8])
                        ots.append(ot)
                out.append(head)

                outpr = outp.rearrange("(mc p) u -> p mc u", p=128)

                def mk_mcol(mcol):
                    def f():
                        pp = mmps.tile([128, PIECE], F32, tag="ps")
                        for s8 in range(NCORES):
                            nc.tensor.matmul(
                                pp[:],
                                wp_sb[:, s8, 128 * mcol:128 * (mcol + 1)],
                                ots[s8][:], start=(s8 == 0),
                                stop=(s8 == NCORES - 1))
                        nc.vector.tensor_scalar_add(stg[:, mcol, :], pp[:],
                                                    bp_sb[:, mcol:mcol + 1])
                        nc.sync.dma_start(
                            outpr[:, mcol:mcol + 1,
                                  PIECE * b:PIECE * (b + 1)],
                            stg[:, mcol:mcol + 1, :])
                    return f

                for mcol in range(NKC):
                    out.append(mk_mcol(mcol))
                return out

            def emit_attn_chunk(b, j, fillers, tiles_left):
                """One tq-chunk of attention, software-pipelined: S(t+1) is
                emitted before O(t) so the PE never queues behind the exp,
                and filler thunks (next batch's qkv, previous batch's proj)
                are drained between tiles to fill remaining bubbles."""
                qT, kT, vT, vh = get_qkv(b)
                o_ps = [ops.tile([65, 512], F32, tag=f"o{h}", name=f"o{h}")
                        for h in range(HL)]
                ktop = 4 * j + 4
                pend = None  # (tk, z, pt) awaiting its O matmuls

                def flush_o():
                    nonlocal pend
                    if pend is None:
                        return
                    tk, z, pt = pend
                    for h in range(HL):
                        nc.tensor.matmul(
                            o_ps[h][0:65, z:512],
                            vh[h][:, 65 * tk:65 * (tk + 1)],
                            pt[:, 512 * h + z:512 * (h + 1)],
                            start=(tk == 0), stop=(tk == ktop - 1))
                    pend = None

                for tk in range(ktop):
                    m = tk - 4 * j
                    # cols [0, z) of this tile are fully causal-masked;
                    # cols [z, z+128) are the triangular block
                    z = 128 * m if m > 0 else 0
                    w = 512 - z
                    s_ps = sps.tile([128, 1024], F32, tag="s_ps")
                    for h in range(HL):
                        base = 512 * h
                        kTs = kT[64 * h:64 * (h + 1),
                                 128 * tk:128 * (tk + 1)]
                        if m >= 0:
                            # mask preload on the triangular block only
                            nc.tensor.matmul(
                                s_ps[:, base + z:base + z + 128],
                                idb[:], tri[:], start=True, stop=False)
                            nc.tensor.matmul(
                                s_ps[:, base + z:base + z + 128],
                                kTs,
                                qT[64 * h:64 * (h + 1),
                                   512 * j + z:512 * j + z + 128],
                                start=False, stop=True)
                            if w > 128:
                                nc.tensor.matmul(
                                    s_ps[:, base + z + 128:base + 512],
                                    kTs,
                                    qT[64 * h:64 * (h + 1),
                                       512 * j + z + 128:512 * (j + 1)],
                                    start=True, stop=True)
                        else:
                            nc.tensor.matmul(
                                s_ps[:, base:base + 512],
                                kTs,
                                qT[64 * h:64 * (h + 1),
                                   512 * j:512 * (j + 1)],
                                start=True, stop=True)
                    pt = ptpool.tile([128, 1024], BF16, tag="pt")
                    if z:
                        exp_src = s_ps[:].rearrange(
                            "p (g c) -> p g c", g=2)[:, :, z:]
                        exp_dst = pt[:].rearrange(
                            "p (g c) -> p g c", g=2)[:, :, z:]
                        nc.scalar.activation(exp_dst, exp_src, Exp)
                    else:
                        nc.scalar.activation(pt[:], s_ps[:], Exp)
                    tiles_left[0] -= 1
                    npop = (len(fillers) + max(tiles_left[0], 1) - 1) \
                        // max(tiles_left[0], 1) if fillers else 0
                    for _ in range(min(npop, len(fillers))):
                        fillers.pop(0)()
                    flush_o()
                    pend = (tk, z, pt)
                flush_o()

                # ---- normalize on the sender with baseline-proven op
                # patterns only: same-partition copies, [1,N] reciprocal,
                # PE ones-matmul broadcast, per-(head,half) stores ----
                for h in range(HL):
                    o_sb = ofinpool.tile([65, 512], F32, tag="osb2")
                    nc.vector.tensor_copy(o_sb[:], o_ps[h][:])
                    r_sb = ofinpool.tile([1, 512], BF16, tag="r")
                    with nc.allow_low_precision(reason="softmax denom"):
                        nc.vector.reciprocal(r_sb[:], o_sb[64:65, :])
                    rb_ps = mmps.tile([64, 512], F32, tag="ps", name="rb_ps")
                    nc.tensor.matmul(rb_ps[:], onesr[:], r_sb[:],
                                     start=True, stop=True)
                    rb_sb = ofinpool.tile([64, 512], F32, tag="rb")
                    nc.vector.tensor_copy(rb_sb[:], rb_ps[:])
                    ofin = ofinpool.tile([64, 512], BF16, tag="ofin")
                    with nc.allow_low_precision(reason="bf16 O"):
                        nc.gpsimd.tensor_mul(ofin[:], o_sb[0:64, :],
                                             rb_sb[:])
                    for half in range(2):
                        s8 = 2 * j + half
                        nc.sync.dma_start(
                            inb[b].ap()[s8, 64 * h:64 * (h + 1), :],
                            ofin[:, 256 * half:256 * (half + 1)])

            def emit_a2a(b):
                if sim:
                    # stand-in with comparable cost for the cost-model sim,
                    # issued from the gpsimd queue like the real collective so
                    # it never head-of-line-blocks the SP DMA queue
                    with tc.high_priority():
                        nc.gpsimd.dma_start(outb[b].ap(), inb[b].ap())
                else:
                    nc.gpsimd.collective_compute(
                        "AllToAll", mybir.AluOpType.bypass,
                        replica_groups=[list(range(NCORES))],
                        ins=[inb[b].ap().opt()], outs=[outb[b].ap().opt()],
                    )

            # prelude: batch 0's qkv runs as a plain phase
            for g in range(NCH):
                for t in qkv_thunks(g):
                    t()

            for b in range(B):
                # filler PE work drained inside this batch's attention:
                # next batch's qkv/v-transposes and batch b-1's projection
                fillers = []
                if b + 1 < B:
                    nxt = [qkv_thunks(NCH * (b + 1) + n) for n in range(NCH)]
                    fillers += nxt[0]
                    if b > 0:
                        fillers += proj_thunks(b - 1)
                    for n in range(1, NCH):
                        fillers += nxt[n]
                elif b > 0:
                    fillers += proj_thunks(b - 1)

                tiles_left = [sum(4 * j + 4 for j in range(NCH))]
                for j in range(NCH):
                    emit_attn_chunk(b, j, fillers, tiles_left)
                for t in fillers:
                    t()
                del fillers[:]
                emit_a2a(b)
                qkv_tiles.pop(b, None)

            for t in proj_thunks(B - 1):
                t()
    nc.compile()
    return nc


def _get_nc():
    if "nc" not in _CACHE:
        _CACHE["nc"] = _build()
    return _CACHE["nc"]


def kernel(x, W_qkv, b_qkv, W_proj, b_proj):
    import ml_dtypes
    bf16 = ml_dtypes.bfloat16

    x = np.asarray(x, dtype=np.float32)
    W_qkv = np.asarray(W_qkv, dtype=np.float32)
    b_qkv = np.asarray(b_qkv, dtype=np.float32)
    W_proj = np.asarray(W_proj, dtype=np.float32)
    b_proj = np.asarray(b_proj, dtype=np.float32)

    scale = 1.0 / np.sqrt(HD)
    xt = np.ascontiguousarray(x.reshape(B * T, C).T).astype(bf16)  # [C, B*T]
    wp = np.ascontiguousarray(W_proj).astype(bf16)                 # [C, C]
    bp = np.ascontiguousarray(b_proj.reshape(NKC, 128).T)          # [128, 8]

    qw = W_qkv[:, 0:C]
    kw = W_qkv[:, C:2 * C]
    vw = W_qkv[:, 2 * C:3 * C]
    qb, kb, vb = b_qkv[0:C], b_qkv[C:2 * C], b_qkv[2 * C:3 * C]

    in_maps = []
    for c in range(NCORES):
        cols = slice(2 * c * HD, (2 * c + 2) * HD)  # this core's 128 dims
        bq = np.stack([qb[cols] * scale, kb[cols], vb[cols]], axis=1)  # [128,3]
        wqkv_c = np.concatenate(
            [qw[:, cols] * scale, kw[:, cols], vw[:, cols]], axis=1)
        in_maps.append({
            "xt": xt,
            "wqkv": np.ascontiguousarray(wqkv_c).astype(bf16),
            "wp": wp,
            "bqkv": np.ascontiguousarray(bq),
            "bp": bp,
        })

    nc = _get_nc()
    _CACHE["last_in_maps"] = in_maps
    res = run_bass_kernel_spmd(nc, in_maps, core_ids=list(range(NCORES)))

    # outp[c]: [C, B*PIECE] (cols: b-major, then 256 tokens of piece c)
    allo = np.stack([res.results[c]["outp"] for c in range(NCORES)])
    allo = allo.reshape(NCORES, C, B, PIECE)       # [c, ch, b, u]
    out = allo.transpose(2, 0, 3, 1).reshape(B, T, C)
    return np.ascontiguousarray(out)
